# revision 1
# baseline (speedup 1.0000x reference)
import sys

sys.path.insert(0, "/opt/trn_rl_repo")

import numpy as np
from numpy.lib.stride_tricks import sliding_window_view

import concourse.bass as bass  # noqa: F401
import concourse.bacc as bacc
import concourse.tile as tile
from concourse import mybir
from concourse.bass_utils import run_bass_kernel_spmd

# Problem geometry (hardcoded per contract)
B, H, W = 8, 1024, 1024
K, S = 16, 8
NH = NW = 127
NWIN = NH * NW          # 16129
NPAD = 16384            # padded window count (32 tiles of 512)
NT = 512
NTILES = NPAD // NT
F32 = mybir.dt.float32

_CACHE = {}


def _build_program():
    nc = bacc.Bacc("TRN2", target_bir_lowering=False, debug=False)

    wt = nc.dram_tensor("wt", [2, 128, NPAD], F32, kind="ExternalInput").ap()
    we = nc.dram_tensor("we", [2, 128, 256], F32, kind="ExternalInput").ap()
    wr = nc.dram_tensor("wr", [2, 128, 256], F32, kind="ExternalInput").ap()
    ws = nc.dram_tensor("ws", [2, 128, 256], F32, kind="ExternalInput").ap()
    wa = nc.dram_tensor("wa", [2, 128, 1], F32, kind="ExternalInput").ap()
    be = nc.dram_tensor("be", [2, 128, 1], F32, kind="ExternalInput").ap()
    br = nc.dram_tensor("br", [2, 128, 1], F32, kind="ExternalInput").ap()
    bs = nc.dram_tensor("bs", [2, 128, 1], F32, kind="ExternalInput").ap()
    ba = nc.dram_tensor("ba", [1, 1], F32, kind="ExternalInput").ap()
    ones = nc.dram_tensor("ones", [1, 128], F32, kind="ExternalInput").ap()
    upd = nc.dram_tensor("upd", [2, 128, NPAD], F32, kind="ExternalOutput").ap()

    AF = mybir.ActivationFunctionType

    with tile.TileContext(nc) as tc:
        with (
            tc.tile_pool(name="const", bufs=1) as cp,
            tc.tile_pool(name="win", bufs=3) as wp,
            tc.tile_pool(name="act", bufs=2) as sp,
            tc.tile_pool(name="psum", bufs=1, space="PSUM") as pp,
        ):
            # Load constants once
            we_sb, wr_sb, ws_sb, wa_sb, be_sb, br_sb, bs_sb = [], [], [], [], [], [], []
            for k in range(2):
                t = cp.tile([128, 256], F32, tag=f"we{k}")
                nc.sync.dma_start(t[:], we[k])
                we_sb.append(t)
                t = cp.tile([128, 256], F32, tag=f"wr{k}")
                nc.sync.dma_start(t[:], wr[k])
                wr_sb.append(t)
                t = cp.tile([128, 256], F32, tag=f"ws{k}")
                nc.sync.dma_start(t[:], ws[k])
                ws_sb.append(t)
                t = cp.tile([128, 1], F32, tag=f"wa{k}")
                nc.sync.dma_start(t[:], wa[k])
                wa_sb.append(t)
                t = cp.tile([128, 1], F32, tag=f"be{k}")
                nc.sync.dma_start(t[:], be[k])
                be_sb.append(t)
                t = cp.tile([128, 1], F32, tag=f"br{k}")
                nc.sync.dma_start(t[:], br[k])
                br_sb.append(t)
                t = cp.tile([128, 1], F32, tag=f"bs{k}")
                nc.sync.dma_start(t[:], bs[k])
                bs_sb.append(t)
            ba_sb = cp.tile([1, 1], F32, tag="ba")
            nc.sync.dma_start(ba_sb[:], ba[:])
            ones_sb = cp.tile([1, 128], F32, tag="ones")
            nc.sync.dma_start(ones_sb[:], ones[:])

            for t in range(NTILES):
                sl = slice(t * NT, (t + 1) * NT)
                w0 = wp.tile([128, NT], F32, tag="w0")
                nc.sync.dma_start(w0[:], wt[0, :, sl])
                w1 = wp.tile([128, NT], F32, tag="w1")
                nc.sync.dma_start(w1[:], wt[1, :, sl])

                # layer 1: expanded.T = We.T @ winf.T ; att_pre = Wa.T @ winf.T
                pe = []
                for m in range(2):
                    p = pp.tile([128, NT], F32, tag=f"pe{m}")
                    ms = slice(m * 128, (m + 1) * 128)
                    nc.tensor.matmul(p[:], we_sb[0][:, ms], w0[:], start=True, stop=False)
                    nc.tensor.matmul(p[:], we_sb[1][:, ms], w1[:], start=False, stop=True)
                    pe.append(p)
                pa = pp.tile([1, NT], F32, tag="pa")
                nc.tensor.matmul(pa[:], wa_sb[0][:], w0[:], start=True, stop=False)
                nc.tensor.matmul(pa[:], wa_sb[1][:], w1[:], start=False, stop=True)

                e = []
                for m in range(2):
                    s = sp.tile([128, NT], F32, tag=f"e{m}")
                    nc.scalar.activation(s[:], pe[m][:], AF.Identity, bias=be_sb[m][:])
                    e.append(s)
                atts = sp.tile([1, NT], F32, tag="atts")
                nc.scalar.activation(atts[:], pa[:], AF.Relu, bias=ba_sb[:])

                # layer 2: rec.T = relu(Wr.T @ expanded.T + br)
                r = []
                for m in range(2):
                    p = pp.tile([128, NT], F32, tag=f"pr{m}")
                    ms = slice(m * 128, (m + 1) * 128)
                    nc.tensor.matmul(p[:], wr_sb[0][:, ms], e[0][:], start=True, stop=False)
                    nc.tensor.matmul(p[:], wr_sb[1][:, ms], e[1][:], start=False, stop=True)
                    s = sp.tile([128, NT], F32, tag=f"r{m}")
                    nc.scalar.activation(s[:], p[:], AF.Relu, bias=br_sb[m][:])
                    r.append(s)

                # broadcast att over 128 partitions via K=1 matmul
                pab = pp.tile([128, NT], F32, tag="pab")
                nc.tensor.matmul(pab[:], ones_sb[:], atts[:], start=True, stop=True)

                # layer 3: rep.T = Ws.T @ rec.T + bs ; upd = rep * att
                for m in range(2):
                    p = pp.tile([128, NT], F32, tag=f"pp{m}")
                    ms = slice(m * 128, (m + 1) * 128)
                    nc.tensor.matmul(p[:], ws_sb[0][:, ms], r[0][:], start=True, stop=False)
                    nc.tensor.matmul(p[:], ws_sb[1][:, ms], r[1][:], start=False, stop=True)
                    rep = sp.tile([128, NT], F32, tag=f"rep{m}")
                    nc.vector.tensor_scalar_add(rep[:], p[:], bs_sb[m][:])
                    u = sp.tile([128, NT], F32, tag=f"u{m}")
                    nc.vector.tensor_mul(u[:], rep[:], pab[:])
                    nc.sync.dma_start(upd[m, :, sl], u[:])

    nc.compile()
    return nc


def _get_nc():
    if "nc" not in _CACHE:
        _CACHE["nc"] = _build_program()
    return _CACHE["nc"]


def kernel(x, Wa, ba, We, be, Wr, br, Ws, bs, current_recursion_floor):
    x = np.asarray(x, dtype=np.float32)
    imgs = x[:, 0]  # (B, H, W)

    # im2col: windows (B, 127, 127, 16, 16) -> winf.T (B, 2, 128, NPAD)
    wins = sliding_window_view(imgs, (K, K), axis=(1, 2))[:, ::S, ::S]
    wt = np.ascontiguousarray(
        wins.transpose(0, 3, 4, 1, 2).reshape(B, 256, NWIN)
    ).astype(np.float32)
    wtp = np.zeros((B, 2, 128, NPAD), np.float32)
    wtp[:, :, :, :NWIN] = wt.reshape(B, 2, 128, NWIN)

    common = {
        "we": np.ascontiguousarray(We, dtype=np.float32).reshape(2, 128, 256),
        "wr": np.ascontiguousarray(Wr, dtype=np.float32).reshape(2, 128, 256),
        "ws": np.ascontiguousarray(Ws, dtype=np.float32).reshape(2, 128, 256),
        "wa": np.ascontiguousarray(Wa, dtype=np.float32).reshape(2, 128, 1),
        "be": np.ascontiguousarray(be, dtype=np.float32).reshape(2, 128, 1),
        "br": np.ascontiguousarray(br, dtype=np.float32).reshape(2, 128, 1),
        "bs": np.ascontiguousarray(bs, dtype=np.float32).reshape(2, 128, 1),
        "ba": np.ascontiguousarray(ba, dtype=np.float32).reshape(1, 1),
        "ones": np.ones((1, 128), np.float32),
    }
    in_maps = [dict(common, wt=wtp[b]) for b in range(B)]

    nc = _get_nc()
    res = run_bass_kernel_spmd(nc, in_maps, core_ids=list(range(B)))
    upd = np.stack([res.results[b]["upd"] for b in range(B)])  # (B,2,128,NPAD)

    # scatter-add of overlapping 16x16 windows, quadrant-decomposed
    u = upd.reshape(B, 256, NPAD)[:, :, :NWIN].reshape(B, K, K, NH, NW)
    out = imgs.copy()
    xb = out.reshape(B, 128, 8, 128, 8)
    for di in (0, 1):
        for dj in (0, 1):
            xb[:, di : di + NH, :, dj : dj + NW, :] += u[
                :, 8 * di : 8 * di + 8, 8 * dj : 8 * dj + 8, :, :
            ].transpose(0, 3, 1, 4, 2)
    return out[:, None].astype(np.float32)

